# revision 2
# baseline (speedup 1.0000x reference)
"""Trainium2 Bass kernel v2 for nn_AttentionLayer1 (LSTM-projected MHA).

LSTM launch (6 cores, one (lstm, batch-half) each, B=16):
  Recurrence in "gT orientation": gates live as [128 gate-rows, (q,c,b)]
  psum tiles; the h @ W_hh matmul uses W_hh chunks as the stationary
  operand ([128,128] bf16) and h^T as the moving operand ([128,16] bf16),
  so per-step PE time is ~64*16 cycles instead of 16*512.
  Gate order in psum: f, i, o, g. gx (input projection + bias) is
  precomputed in phase A as bf16 and either added on DVE ('dve' mode) or
  matmul-injected into psum via a [128,128] identity ('idmm' mode).
  The cell tail runs on ACT/DVE/Pool with everything transposed
  ([hidden-on-partitions, batch]) so no per-step transpose is needed:
  h^T feeds the next step's matmul directly.

Attention launch (8 cores, 4 batches each): unchanged from v1.
"""

import functools

import ml_dtypes
import numpy as np

import concourse.bacc as bacc
import concourse.mybir as mybir
from concourse import bass_utils
from concourse.masks import make_identity
from concourse.tile import TileContext

F32 = mybir.dt.float32
F32R = mybir.dt.float32r
BF16 = mybir.dt.bfloat16
BF16NP = ml_dtypes.bfloat16

B = 32
BH = 16
L = 512
D = 512
G = 4 * D
NH = 8
HD = D // NH
NCORES = 8

# PyTorch gate row order is (i, f, g, o); we lay psum out as (f, i, o, g).
GATE_PERM = [1, 0, 3, 2]

INIT_MODE = "idmm"  # 'idmm' (identity matmul psum init) or 'dve' (vector add)


# ----------------------------------------------------------------- launch 1
def build_lstm_program(steps=L, init_mode=INIT_MODE):
    TOK = steps * BH
    NM = TOK // 512
    GWIN = min(32, steps)
    WIN = min(64, steps)
    idmm = init_mode == "idmm"

    nc = bacc.Bacc("TRN2", target_bir_lowering=False, debug=False)
    xT = nc.dram_tensor("xT", [4, 128, TOK], BF16, kind="ExternalInput").ap()
    wih = nc.dram_tensor("wih", [4, 128, G], BF16, kind="ExternalInput").ap()
    whh = nc.dram_tensor("whh", [4, 128, G], BF16, kind="ExternalInput").ap()
    biasd = nc.dram_tensor("biasd", [128, 16], F32, kind="ExternalInput").ap()
    hh_out = nc.dram_tensor(
        "hh_out", [4, 128, BH, steps], BF16, kind="ExternalOutput"
    ).ap()

    Act = mybir.ActivationFunctionType
    with TileContext(nc) as tc:
        with tc.tile_pool(name="consts", bufs=1) as cp:
            wih_sb, whh_sb = [], []
            for k in range(4):
                wi_t = cp.tile([128, G], BF16, tag=f"wih{k}", name=f"wih{k}")
                nc.sync.dma_start(out=wi_t[:, :], in_=wih[k])
                wih_sb.append(wi_t)
                wh_t = cp.tile([128, G], BF16, tag=f"whh{k}", name=f"whh{k}")
                nc.sync.dma_start(out=wh_t[:, :], in_=whh[k])
                whh_sb.append(wh_t)
            bias_sb = cp.tile([128, 16], F32, tag="bias")
            nc.sync.dma_start(out=bias_sb[:, :], in_=biasd[:, :])
            id128 = cp.tile([128, 128], BF16, tag="id128")
            make_identity(nc, id128[:, :])
            h0 = cp.tile([128, 4, BH], BF16, tag="h0")
            nc.vector.memset(h0[:, :, :], 0.0)
            # CG = [C | G]: C persistent cell state, G = tanh gate scratch
            CG = cp.tile([128, 8, BH], F32, tag="CG")
            nc.vector.memset(CG[:, :, :], 0.0)

            with tc.tile_pool(name="gxdram", bufs=1, space="DRAM") as dp:
                gxd = dp.tile([16, 128, TOK], BF16)

                # ---- phase A: gx = x @ w_ih.T + bias  (bf16, gT layout)
                with (
                    tc.tile_pool(name="pa_x", bufs=3) as xp,
                    tc.tile_pool(name="pa_ps", bufs=4, space="PSUM") as pap,
                    tc.tile_pool(name="pa_o", bufs=4) as gop,
                ):
                    for m in range(NM):
                        xa = xp.tile([128, 4, 512], BF16, tag="xa")
                        for kx in range(4):
                            nc.sync.dma_start(
                                out=xa[:, kx, :],
                                in_=xT[kx, :, m * 512:(m + 1) * 512],
                            )
                        for qc in range(16):
                            ps = pap.tile([128, 512], F32, tag="pa")
                            for kx in range(4):
                                nc.tensor.matmul(
                                    ps[:, :],
                                    wih_sb[kx][:, qc * 128:(qc + 1) * 128],
                                    xa[:, kx, :],
                                    start=(kx == 0), stop=(kx == 3),
                                )
                            go = gop.tile([128, 512], BF16, tag="go")
                            nc.vector.tensor_scalar_add(
                                go[:, :], ps[:, :], bias_sb[:, qc:qc + 1]
                            )
                            nc.sync.dma_start(
                                out=gxd[qc, :, m * 512:(m + 1) * 512],
                                in_=go[:, :],
                            )

                # ---- phase B: the recurrence
                with (
                    tc.tile_pool(name="gw", bufs=2) as gwp,
                    tc.tile_pool(name="hh", bufs=2) as hhp,
                    tc.tile_pool(name="pb", bufs=2, space="PSUM") as pbp,
                    tc.tile_pool(name="wk", bufs=3) as wp,
                ):
                    NGW = steps // GWIN

                    def load_gw(w):
                        gwt = gwp.tile(
                            [128, 16, GWIN * 16], BF16, tag="gw", name=f"gw{w}"
                        )
                        span = slice(w * GWIN * 16, (w + 1) * GWIN * 16)
                        for j in range(8):
                            src = gxd[2 * j:2 * j + 2, :, span]
                            nc.sync.dma_start(
                                out=gwt[:, 2 * j:2 * j + 2, :],
                                in_=src.transpose([1, 0, 2]),
                            )
                        return gwt

                    gw_cur = load_gw(0)
                    gw_next = None
                    hht = None
                    hprev = h0  # [128, 4, BH] view of previous step's h^T
                    for t in range(steps):
                        w, wi = divmod(t, GWIN)
                        hw_, hwi = divmod(t, WIN)
                        if wi == 1 and w + 1 < NGW:
                            gw_next = load_gw(w + 1)
                        if wi == 0 and w > 0:
                            gw_cur = gw_next
                        if hwi == 0:
                            hht = hhp.tile(
                                [128, 4, BH, WIN], BF16, tag="hh", name=f"hh{hw_}"
                            )
                        gws = gw_cur[:, :, wi * 16:(wi + 1) * 16]  # [128,16qc,16b]
                        ps = pbp.tile([128, 16, BH], F32, tag="pb")
                        if idmm:
                            nc.tensor.matmul(
                                ps[:, :, :], id128[:, :], gws,
                                start=True, stop=False, skip_group_check=True,
                            )
                        # emit g's groups first so tanh-g starts early
                        qc_order = ((12, 13, 14, 15, 0, 1, 2, 3, 4, 5, 6, 7,
                                     8, 9, 10, 11) if idmm else range(16))
                        for qc in qc_order:  # f:0-3 i:4-7 o:8-11 g:12-15
                            for kc in range(4):
                                nc.tensor.matmul(
                                    ps[:, qc, :],
                                    whh_sb[kc][:, qc * 128:(qc + 1) * 128],
                                    hprev[:, kc, :],
                                    start=(False if idmm else kc == 0),
                                    stop=(kc == 3),
                                    skip_group_check=idmm,
                                )
                        U = wp.tile([128, 12, BH], F32, tag="U")
                        if idmm:
                            nc.scalar.activation(
                                CG[:, 4:8, :], ps[:, 12:16, :], Act.Tanh
                            )
                            nc.scalar.activation(
                                U[:, 0:8, :], ps[:, 0:8, :], Act.Sigmoid
                            )
                            nc.scalar.activation(
                                U[:, 8:12, :], ps[:, 8:12, :], Act.Sigmoid
                            )
                        else:
                            V1 = wp.tile([128, 12, BH], F32, tag="V1")
                            nc.vector.tensor_add(
                                V1[:, :, :], ps[:, 0:12, :], gws[:, 0:12, :]
                            )
                            nc.scalar.activation(U[:, :, :], V1[:, :, :], Act.Sigmoid)
                            V2 = wp.tile([128, 4, BH], F32, tag="V2")
                            nc.vector.tensor_add(
                                V2[:, :, :], ps[:, 12:16, :], gws[:, 12:16, :]
                            )
                            nc.scalar.activation(CG[:, 4:8, :], V2[:, :, :], Act.Tanh)
                        # T = [F|I] * [C|G]; C' = T0 + T1; h = O * tanh(C')
                        T = wp.tile([128, 8, BH], F32, tag="T")
                        nc.vector.tensor_mul(T[:, :, :], U[:, 0:8, :], CG[:, :, :])
                        nc.vector.tensor_add(
                            CG[:, 0:4, :], T[:, 0:4, :], T[:, 4:8, :]
                        )
                        TH = wp.tile([128, 4, BH], F32, tag="TH")
                        nc.scalar.activation(TH[:, :, :], CG[:, 0:4, :], Act.Tanh)
                        nc.vector.tensor_mul(
                            hht[:, :, :, hwi], U[:, 8:12, :], TH[:, :, :]
                        )
                        hprev = hht[:, :, :, hwi]
                        if hwi == WIN - 1:
                            for c in range(4):
                                nc.sync.dma_start(
                                    out=hh_out[c, :, :, hw_ * WIN:(hw_ + 1) * WIN],
                                    in_=hht[:, c, :, :],
                                )
    nc.finalize()
    return nc


# ----------------------------------------------------------------- launch 2
def build_attn_program():
    BL = 4  # batch per core
    nc = bacc.Bacc("TRN2", target_bir_lowering=False, debug=False)
    qT = nc.dram_tensor("qT", [BL, D, L], F32R, kind="ExternalInput").ap()
    kT = nc.dram_tensor("kT", [BL, D, L], F32R, kind="ExternalInput").ap()
    vn = nc.dram_tensor("vn", [BL, L, D], F32R, kind="ExternalInput").ap()
    w_outT = nc.dram_tensor("w_outT", [D, D], F32R, kind="ExternalInput").ap()
    b_outc = nc.dram_tensor("b_outc", [128, 4], F32, kind="ExternalInput").ap()
    ones_r = nc.dram_tensor("ones_r", [1, 128], F32R, kind="ExternalInput").ap()
    ones_c = nc.dram_tensor("ones_c", [128, 1], F32R, kind="ExternalInput").ap()
    outT = nc.dram_tensor("outT", [BL, D, L], F32, kind="ExternalOutput").ap()

    Act = mybir.ActivationFunctionType
    with TileContext(nc) as tc, nc.allow_low_precision("softmax recip to f32r"):
        with tc.tile_pool(name="consts", bufs=1) as cp:
            w_sb = []
            for k in range(4):
                w = cp.tile([128, D], F32R, tag=f"wo{k}", name=f"wo{k}")
                nc.sync.dma_start(out=w[:, :], in_=w_outT[k * 128:(k + 1) * 128, :])
                w_sb.append(w)
            b_sb = cp.tile([128, 4], F32, tag="bo")
            nc.sync.dma_start(out=b_sb[:, :], in_=b_outc[:, :])
            ones1 = cp.tile([1, 128], F32R, tag="ones1")
            nc.sync.dma_start(out=ones1[:, :], in_=ones_r[:, :])
            onescol = cp.tile([128, 1], F32R, tag="onescol")
            nc.sync.dma_start(out=onescol[:, :], in_=ones_c[:, :])

            with (
                tc.tile_pool(name="inq", bufs=2) as qp,
                tc.tile_pool(name="ink", bufs=2) as kp,
                tc.tile_pool(name="inv", bufs=2) as vp,
                tc.tile_pool(name="Epool", bufs=8) as ep,
                tc.tile_pool(name="attn", bufs=2) as ap_,
                tc.tile_pool(name="rsb", bufs=2) as rp,
                tc.tile_pool(name="osb", bufs=3) as op_,
                tc.tile_pool(name="ps_big", bufs=4, space="PSUM") as psb,
                tc.tile_pool(name="ps_s", bufs=1, space="PSUM") as pss,
                tc.tile_pool(name="ps_r", bufs=1, space="PSUM") as psr,
                tc.tile_pool(name="ps_o", bufs=2, space="PSUM") as pso,
            ):
                for b in range(BL):
                    q_sb, k_sb, v_sb = [], [], []
                    for k in range(4):
                        qt = qp.tile([128, L], F32R, tag=f"q{k}", name=f"q{k}")
                        nc.sync.dma_start(out=qt[:, :], in_=qT[b, k * 128:(k + 1) * 128, :])
                        q_sb.append(qt)
                        kt = kp.tile([128, L], F32R, tag=f"k{k}", name=f"k{k}")
                        nc.sync.dma_start(out=kt[:, :], in_=kT[b, k * 128:(k + 1) * 128, :])
                        k_sb.append(kt)
                        vt = vp.tile([128, D], F32R, tag=f"v{k}", name=f"v{k}")
                        nc.sync.dma_start(out=vt[:, :], in_=vn[b, k * 128:(k + 1) * 128, :])
                        v_sb.append(vt)
                    at_sb = [
                        ap_.tile([128, L], F32R, tag=f"at{k}", name=f"at{k}")
                        for k in range(4)
                    ]
                    for h in range(NH):
                        ct, ro = h // 2, (h % 2) * HD
                        E = []
                        for kc in range(4):
                            ps = psb.tile([128, L], F32, tag="big", name="ps_sc")
                            nc.tensor.matmul(
                                ps[:, :],
                                k_sb[ct][ro:ro + HD, kc * 128:(kc + 1) * 128],
                                q_sb[ct][ro:ro + HD, :],
                                start=True, stop=True,
                            )
                            e = ep.tile([128, L], F32R, tag="E", name="E")
                            nc.scalar.activation(e[:, :], ps[:, :], Act.Exp, scale=0.125)
                            E.append(e)
                        ps_s = pss.tile([1, L], F32, tag="s")
                        for kc in range(4):
                            nc.tensor.matmul(
                                ps_s[:, :], onescol[:, :], E[kc][:, :],
                                start=(kc == 0), stop=(kc == 3),
                            )
                        r_sb = rp.tile([1, L], F32R, tag="r")
                        nc.vector.reciprocal(r_sb[:, :], ps_s[:, :])
                        ps_r = psr.tile([128, L], F32, tag="R")
                        nc.tensor.matmul(
                            ps_r[:, :], ones1[:1, :], r_sb[:1, :],
                            start=True, stop=True,
                        )
                        R_sb = rp.tile([128, L], F32, tag="Rsb")
                        nc.vector.tensor_copy(R_sb[:, :], ps_r[:, :])
                        ps_o = pso.tile([HD, L], F32, tag="o")
                        for kc in range(4):
                            nc.tensor.matmul(
                                ps_o[:, :],
                                v_sb[kc][:, h * HD:(h + 1) * HD],
                                E[kc][:, :],
                                start=(kc == 0), stop=(kc == 3),
                            )
                        nc.vector.tensor_mul(
                            at_sb[ct][ro:ro + HD, :], ps_o[:, :], R_sb[:HD, :]
                        )
                    for oc in range(4):
                        ps = psb.tile([128, L], F32, tag="big", name="ps_pj")
                        for k in range(4):
                            nc.tensor.matmul(
                                ps[:, :],
                                w_sb[k][:, oc * 128:(oc + 1) * 128],
                                at_sb[k][:, :],
                                start=(k == 0), stop=(k == 3),
                            )
                        o_sb = op_.tile([128, L], F32, tag="osb")
                        nc.vector.tensor_scalar_add(o_sb[:, :], ps[:, :], b_sb[:, oc:oc + 1])
                        nc.sync.dma_start(
                            out=outT[b, oc * 128:(oc + 1) * 128, :], in_=o_sb[:, :]
                        )
    nc.finalize()
    return nc


@functools.lru_cache(maxsize=1)
def _programs():
    return build_lstm_program(), build_attn_program()


def _prep_lstm_inputs(x, w_ih, w_hh, b_ih, b_hh, steps=L):
    """Host-side input prep for one (lstm, batch-half) core."""
    nb = x.shape[0]
    # xT[kx, p, s*nb+b] = x[b, s, kx*128+p]
    xT = np.ascontiguousarray(
        x.transpose(2, 1, 0).reshape(4, 128, steps * nb).astype(BF16NP))

    def permg(w):  # [4H, ...] torch gate rows (i,f,g,o) -> (f,i,o,g)
        blocks = [w[512 * p:512 * (p + 1)] for p in GATE_PERM]
        return np.concatenate(blocks, axis=0)

    wihp = np.ascontiguousarray(
        permg(w_ih).T.reshape(4, 128, G).astype(BF16NP))
    whhp = np.ascontiguousarray(
        permg(w_hh).T.reshape(4, 128, G).astype(BF16NP))
    biasp = np.ascontiguousarray(
        permg((b_ih + b_hh).astype(np.float32)).reshape(16, 128).T)
    return {"xT": xT, "wih": wihp, "whh": whhp, "biasd": biasp}


def kernel(query, key, value,
           w_ih_q, w_hh_q, b_ih_q, b_hh_q,
           w_ih_k, w_hh_k, b_ih_k, b_hh_k,
           w_ih_v, w_hh_v, b_ih_v, b_hh_v,
           w_out, b_out, _trace=False, _results=None):
    nc1, nc2 = _programs()
    xs = {'q': query, 'k': key, 'v': value}
    ws = {
        'q': (w_ih_q, w_hh_q, b_ih_q, b_hh_q),
        'k': (w_ih_k, w_hh_k, b_ih_k, b_hh_k),
        'v': (w_ih_v, w_hh_v, b_ih_v, b_hh_v),
    }
    # ---- launch 1: 6 cores, (q|k|v) x (batch half)
    in_maps1 = []
    for c in range(6):
        name = 'qkv'[c // 2]
        half = c % 2
        x = np.ascontiguousarray(xs[name][half * BH:(half + 1) * BH])
        w_ih, w_hh, b_ih, b_hh = ws[name]
        in_maps1.append(_prep_lstm_inputs(x, w_ih, w_hh, b_ih, b_hh))
    res1 = bass_utils.run_bass_kernel_spmd(
        nc1, in_maps1, core_ids=list(range(6)), trace=_trace)
    if _results is not None:
        _results.append(res1)
    # hh_out [4, 128, BH, L] -> hd [512 d, BH, L]
    hs = {}
    for i, name in enumerate('qkv'):
        halves = [res1.results[2 * i + h]['hh_out'].reshape(D, BH, L)
                  for h in range(2)]
        hs[name] = np.concatenate(halves, axis=1)  # [D, B, L] bf16
    # ---- launch 2
    w_outT = np.ascontiguousarray(w_out.T.astype(np.float32))
    b_outc = np.ascontiguousarray(
        b_out.astype(np.float32).reshape(4, 128).T)
    ones_r = np.ones((1, 128), np.float32)
    ones_c = np.ones((128, 1), np.float32)
    qT_all = hs['q'].transpose(1, 0, 2).astype(np.float32)   # [B, D, L]
    kT_all = hs['k'].transpose(1, 0, 2).astype(np.float32)
    vn_all = hs['v'].transpose(1, 2, 0).astype(np.float32)   # [B, L, D]
    in_maps2 = []
    for c in range(NCORES):
        bs = slice(4 * c, 4 * c + 4)
        in_maps2.append({
            'qT': np.ascontiguousarray(qT_all[bs]),
            'kT': np.ascontiguousarray(kT_all[bs]),
            'vn': np.ascontiguousarray(vn_all[bs]),
            'w_outT': w_outT,
            'b_outc': b_outc,
            'ones_r': ones_r,
            'ones_c': ones_c,
        })
    res2 = bass_utils.run_bass_kernel_spmd(
        nc2, in_maps2, core_ids=list(range(NCORES)), trace=_trace)
    if _results is not None:
        _results.append(res2)
    out = np.concatenate(
        [res2.results[c]['outT'].transpose(0, 2, 1) for c in range(NCORES)],
        axis=0)
    return out.astype(np.float32)
